# revision 1
# baseline (speedup 1.0000x reference)
"""DecoderTreeRNN Trainium2 kernel (8 NeuronCores, single SPMD launch).

  - Tree expansion: data-parallel over batch B (8 examples/core). GRU states
    kept transposed [H, nodes] in bf16; each level is ghT = WhhT.T @ hT with
    fp8(e4m3) weight tiles stationary on the PE (fp8 FWL makes the weight
    load, the tree's floor, 2-4x faster; states stay bf16). Gate biases are
    folded in with free-dim-broadcast adds on VectorE; sigmoid/tanh run on
    ScalarE from one ACT table set, all on 4-wide m-tile slabs. Children are
    concatenated [left | right]; the bit-reversed leaf order is undone on
    the host during unshard. The last level writes fp8 states directly.
  - The fp8 leaf states are AllGathered so every core holds all B*32 rows.
  - Output projection: tensor-parallel over vocab (4000 columns/core),
    fp8 DoubleRow matmuls (K=256 per tile, pre-paired k=256*k2+128*j+p
    layout on both operands). Per row tile one stationary leaf tile serves
    all 8 vocab chunks, each accumulating in its own PSUM bank. The f32
    vocab bias is added during the PSUM->SBUF copy (VectorE) and exp +
    row-sum is fused on ScalarE via accum_out. Unnormalized logits stream
    straight out; each core also returns its per-row exp-sums and
    -log(sum over cores) is folded into the host-side unshard pass.
  DMA discipline: the two HWDGE rings (SP + ACT) are ordered FIFOs - small
  latency-critical inputs and right-side tree weights on the ACT ring,
  left-side tree weights then projection weights on the SP ring.
"""

import sys

for _p in ("/opt/trn_rl_repo",):
    if _p not in sys.path:
        sys.path.append(_p)

import numpy as np
import ml_dtypes

import concourse.bass as bass
from concourse import bacc, tile, mybir
from concourse import bass_utils
from contextlib import ExitStack

BF16 = mybir.dt.bfloat16
F32 = mybir.dt.float32
AF = mybir.ActivationFunctionType
ALU = mybir.AluOpType
BFNP = ml_dtypes.bfloat16
FP8 = mybir.dt.float8e4
FP8_AG = True   # leaves in fp8: feeds the DoubleRow projection

N_CORES = 8
CW = 500  # vocab chunk width (<=512 fp32 psum bank)


def _build(B, H, V, DEPTH):
    KT = H // 128            # contraction tiles
    MT = 3 * KT              # output m-tiles per GRU side
    Bl = B // N_CORES        # examples per core
    L = 1 << DEPTH           # leaves per example
    NLOC = Bl * L            # local leaf count
    ROWS = B * L             # total leaf rows
    RT = ROWS // 128         # row tiles
    Vs = V // N_CORES        # vocab shard
    NCH = Vs // CW           # chunks per shard
    SG = min(4, KT)          # m-tiles per gate slab
    NSL = KT // SG           # slabs per gate
    assert B % N_CORES == 0 and H % 128 == 0 and V % N_CORES == 0
    assert Vs % CW == 0 and ROWS % 128 == 0 and RT <= 512
    assert SG * 128 <= 512  # psum slab fits one bank

    nc = bacc.Bacc("TRN2", target_bir_lowering=False, debug=False,
                   num_devices=N_CORES, dynamic_dma_scratch_size=2048)

    # ---------------- DRAM I/O ----------------
    encT = nc.dram_tensor("encT", [H, Bl], BF16, kind="ExternalInput")
    wt_d, wb_d, bih2_d = {}, {}, {}
    for s in "lr":
        wt_d[s] = nc.dram_tensor(f"wt_{s}", [H, 3 * H], FP8, kind="ExternalInput")
        wb_d[s] = nc.dram_tensor(f"wb_{s}", [128, 3 * KT], F32, kind="ExternalInput")
        bih2_d[s] = nc.dram_tensor(f"bih2_{s}", [128, KT], F32,
                                   kind="ExternalInput")
    KT2 = KT // 2            # DoubleRow k-tiles (K=256 each)
    woT_d = nc.dram_tensor("woT", [128, KT2, 2, Vs], FP8, kind="ExternalInput")
    bo_d = nc.dram_tensor("bo", [128, Vs], F32, kind="ExternalInput")
    out_d = nc.dram_tensor("out", [ROWS, Vs], F32, kind="ExternalOutput")

    AGDT = FP8 if FP8_AG else BF16
    ag_leaves = nc.dram_tensor("ag_leaves", [N_CORES * H, NLOC], AGDT,
                               kind="Internal", addr_space="Shared")
    s_out_d = nc.dram_tensor("s_out", [128, RT], F32, kind="ExternalOutput")

    rg = [list(range(N_CORES))]

    with tile.TileContext(nc) as tc, ExitStack() as ctx:
        dram = ctx.enter_context(tc.tile_pool(name="dram", bufs=1, space="DRAM"))
        wproj = ctx.enter_context(tc.tile_pool(name="wproj", bufs=1))
        cpool = ctx.enter_context(tc.tile_pool(name="const", bufs=1))

        # projection weights: resident for the whole kernel. Tiles are
        # allocated up front but their DMAs are issued after the tree weight
        # DMAs (below) so the tree isn't starved of HBM bandwidth at start.
        wo_sb = wproj.tile([128, KT2, 2, Vs], FP8, tag="wo8", name="wo8")
        bo_sb = cpool.tile([128, Vs], F32, tag="bo")
        ones_sb = cpool.tile([1, 128], BF16, tag="ones")
        nc.vector.memset(ones_sb[:], 1.0)

        leaves_bounce = dram.tile([H, NLOC], AGDT, tag="lvb")

        # ---------------- tree expansion ----------------
        with nc.named_scope("tree"):
            with tc.tile_pool(name="wtree", bufs=1) as wtp, \
                 tc.tile_pool(name="state", bufs=2) as stp, \
                 tc.tile_pool(name="gates", bufs=2) as gp, \
                 tc.tile_pool(name="pstree", bufs=8, space="PSUM") as pst:
                # latency-critical small inputs go on the ACT HWDGE ring so
                # they aren't stuck behind the big weight loads (SP ring FIFO)
                cur = stp.tile([128, KT, Bl], BF16, tag="st")
                nc.scalar.dma_start(cur[:], encT.ap().rearrange("(k p) b -> p k b", k=KT))
                wt_sb, wb_sb, bih2_sb = {}, {}, {}
                for s in "lr":
                    wb_sb[s] = wtp.tile([128, 3 * KT], F32, tag=f"wb{s}", name=f"wb_sb_{s}")
                    nc.scalar.dma_start(wb_sb[s][:], wb_d[s].ap())
                    bih2_sb[s] = wtp.tile([128, KT], F32, tag=f"bi{s}", name=f"bih2_sb_{s}")
                    nc.scalar.dma_start(bih2_sb[s][:], bih2_d[s].ap())
                # weight loads in consumption order: side l, side r, then the
                # projection weights behind them (all FIFO on the SP ring)
                for s in "lr":
                    eng = nc.sync if s == "l" else nc.scalar
                    wt_sb[s] = []
                    for k in range(KT):
                        t = wtp.tile([128, 3 * H], FP8, tag=f"wt{s}{k}")
                        eng.dma_start(t[:], wt_d[s].ap()[128 * k:128 * (k + 1), :])
                        wt_sb[s].append(t)
                nc.sync.dma_start(wo_sb[:], woT_d.ap())
                nc.sync.dma_start(bo_sb[:], bo_d.ap())

                n = Bl
                for lvl in range(DEPTH):
                    last = lvl == DEPTH - 1
                    nxt = stp.tile([128, KT, 2 * n], AGDT if last else BF16,
                                   tag="st8" if last else "st",
                                   name=f"nxt{lvl}", bufs=1 if last else None)
                    for si, s in enumerate("lr"):
                        for sl in range(NSL):
                            ko0 = sl * SG
                            ps = {}
                            for gi, mb in (("r", ko0), ("z", KT + ko0), ("g", 2 * KT + ko0)):
                                p = pst.tile([128, SG, n], F32, tag="ps")
                                for mj in range(SG):
                                    m = mb + mj
                                    for k in range(KT):
                                        nc.tensor.matmul(
                                            p[:, mj, :],
                                            wt_sb[s][k][:, 128 * m:128 * (m + 1)],
                                            cur[:, k, :n],
                                            start=(k == 0), stop=(k == KT - 1))
                                ps[gi] = p
                            # biases folded in via free-dim-broadcast adds (DVE)
                            def _bias(mb_):
                                return wb_sb[s][:, mb_:mb_ + SG].unsqueeze(2)                                    .broadcast_to((128, SG, n))
                            y_r = gp.tile([128, SG, n], F32, tag="yr")
                            nc.vector.tensor_tensor(y_r[:], ps["r"][:], _bias(ko0), op=ALU.add)
                            r_t = gp.tile([128, SG, n], F32, tag="r")
                            nc.scalar.activation(r_t[:], y_r[:], AF.Sigmoid)
                            y_z = gp.tile([128, SG, n], F32, tag="yz")
                            nc.vector.tensor_tensor(y_z[:], ps["z"][:], _bias(KT + ko0), op=ALU.add)
                            z_t = gp.tile([128, SG, n], F32, tag="z")
                            nc.scalar.activation(z_t[:], y_z[:], AF.Sigmoid)
                            y_g = gp.tile([128, SG, n], F32, tag="yg")
                            nc.vector.tensor_tensor(y_g[:], ps["g"][:], _bias(2 * KT + ko0), op=ALU.add)
                            t_t = gp.tile([128, SG, n], F32, tag="t")
                            nc.vector.tensor_tensor(t_t[:], y_g[:], r_t[:], op=ALU.mult)
                            nc.vector.tensor_tensor(
                                t_t[:], t_t[:],
                                bih2_sb[s][:, ko0:ko0 + SG].unsqueeze(2)
                                .broadcast_to((128, SG, n)), op=ALU.add)
                            n_t = gp.tile([128, SG, n], F32, tag="n")
                            nc.scalar.activation(n_t[:], t_t[:], AF.Tanh)
                            u_t = gp.tile([128, SG, n], F32, tag="u")
                            nc.vector.scalar_tensor_tensor(
                                u_t[:], n_t[:], -1.0, cur[:, ko0:ko0 + SG, :n],
                                op0=ALU.mult, op1=ALU.add)  # u = h - n
                            nc.vector.tensor_tensor(u_t[:], u_t[:], z_t[:], op=ALU.mult)
                            nc.vector.tensor_tensor(
                                nxt[:, ko0:ko0 + SG, si * n:si * n + n],
                                u_t[:], n_t[:], op=ALU.add)
                    cur = nxt
                    n *= 2

                for k in range(KT):
                    eng = nc.sync if k % 2 == 0 else nc.scalar
                    eng.dma_start(leaves_bounce[128 * k:128 * (k + 1), :],
                                  cur[:, k, :])

        # ---------------- leaves all-gather ----------------
        with nc.named_scope("ag_leaves"):
            nc.gpsimd.collective_compute(
                "AllGather", ALU.bypass, replica_groups=rg,
                ins=[leaves_bounce.opt()], outs=[ag_leaves.ap()])

        # ---------------- projection + log-softmax ----------------
        with nc.named_scope("proj"):
            with tc.tile_pool(name="leaves", bufs=1) as lvp, \
                 tc.tile_pool(name="logits", bufs=3) as lgp, \
                 tc.tile_pool(name="scr", bufs=4) as scp, \
                 tc.tile_pool(name="stats", bufs=2) as sp2, \
                 tc.tile_pool(name="psproj", bufs=8, space="PSUM") as psp:
                ag_view = ag_leaves.ap().rearrange("(c h) j -> h c j", c=N_CORES)
                lvbig = lvp.tile([128, KT, N_CORES * NLOC], AGDT, tag="lvbig")
                for k in range(KT):
                    eng = nc.sync if k % 2 == 0 else nc.scalar
                    eng.dma_start(
                        lvbig[:, k, :].rearrange("p (c j) -> p c j", c=N_CORES),
                        ag_view[128 * k:128 * (k + 1)])

                # unnormalized logits stream out as soon as each row tile is
                # done; the per-shard softmax denominators are returned as a
                # tiny second output and log(sum) is folded into the host-side
                # unshard pass.
                s_all = sp2.tile([128, RT], F32, tag="sall", name="s_all")
                for r in range(RT):
                    lg = lgp.tile([128, Vs], F32, tag="lg", name=f"lg{r}")
                    sp = sp2.tile([128, NCH], F32, tag="spart", name=f"sp{r}")
                    # k-outer so one stationary (leaves) tile serves all NCH
                    # chunks; each chunk accumulates in its own PSUM bank
                    pps = [psp.tile([128, CW], F32, tag="pp", name=f"pp{r}_{nch}")
                           for nch in range(NCH)]
                    for k2 in range(KT2):
                        lhsT = lvbig[:, 2 * k2:2 * k2 + 2, 128 * r:128 * (r + 1)]
                        for nch in range(NCH):
                            nc.tensor.matmul(
                                pps[nch][:], lhsT,
                                wo_sb[:, k2, :, CW * nch:CW * (nch + 1)],
                                perf_mode=mybir.MatmulPerfMode.DoubleRow,
                                start=(k2 == 0), stop=(k2 == KT2 - 1))
                    for nch in range(NCH):
                        # bias add fused into the PSUM->SBUF copy
                        nc.vector.tensor_tensor(
                            lg[:, CW * nch:CW * (nch + 1)], pps[nch][:],
                            bo_sb[:, CW * nch:CW * (nch + 1)],
                            op=ALU.add)
                        ex = scp.tile([128, CW], BF16, tag="exp",
                                      name=f"ex{r}_{nch}")
                        nc.scalar.activation(ex[:],
                                             lg[:, CW * nch:CW * (nch + 1)],
                                             AF.Exp,
                                             accum_out=sp[:, nch:nch + 1])
                    nc.vector.reduce_sum(s_all[:, r:r + 1], sp[:],
                                         axis=mybir.AxisListType.X)
                    nc.sync.dma_start(out_d.ap()[128 * r:128 * (r + 1), :], lg[:])
                nc.scalar.dma_start(s_out_d.ap()[:, :], s_all[:])

    nc.compile()
    return nc


_CACHE = {}


def _get(B, H, V, DEPTH):
    key = (B, H, V, DEPTH)
    if key not in _CACHE:
        _CACHE[key] = _build(B, H, V, DEPTH)
    return _CACHE[key]


def _pack_inputs(B, H, V, DEPTH, encoding, Whh_l, bih_l, bhh_l, Whh_r, bih_r,
                 bhh_r, W_out, b_out):
    """Host-side shard + transpose + cast. Returns in_maps for the 8 cores."""
    KT = H // 128
    Bl = B // N_CORES
    Vs = V // N_CORES

    KT2 = KT // 2
    woT = np.ascontiguousarray(W_out.T).astype(np.float32)    # [H, V]
    encT = np.ascontiguousarray(encoding.T).astype(BFNP)      # [H, B]

    shared = {}
    for s, Whh, bih, bhh in (("l", Whh_l, bih_l, bhh_l), ("r", Whh_r, bih_r, bhh_r)):
        shared[f"wt_{s}"] = np.ascontiguousarray(Whh.T).astype(
            mybir.dt.np(FP8))  # [H, 3H] fp8: weight-load bound, not precision bound
        # bias row folded into the matmul: sigmoid gates get bih+bhh,
        # candidate gate gets bhh only (bih_n is added after the r-multiply)
        wb = np.concatenate([(bih + bhh)[:2 * H], bhh[2 * H:]])
        shared[f"wb_{s}"] = np.ascontiguousarray(
            wb.reshape(3 * KT, 128).T.astype(np.float32))
        shared[f"bih2_{s}"] = np.ascontiguousarray(
            bih[2 * H:].reshape(KT, 128).T.astype(np.float32))  # [128, KT]

    in_maps = []
    for c in range(N_CORES):
        m = dict(shared)
        m["encT"] = np.ascontiguousarray(encT[:, c * Bl:(c + 1) * Bl])
        w = woT[:, c * Vs:(c + 1) * Vs].reshape(KT2, 2, 128, Vs)
        m["woT"] = np.ascontiguousarray(
            w.transpose(2, 0, 1, 3)).astype(mybir.dt.np(FP8))
        m["bo"] = np.ascontiguousarray(np.broadcast_to(
            b_out[c * Vs:(c + 1) * Vs].astype(np.float32), (128, Vs)))
        in_maps.append(m)
    return in_maps


def _run(B, H, V, DEPTH, inputs, trace=False, nc=None):
    if nc is None:
        nc = _get(B, H, V, DEPTH)
    in_maps = _pack_inputs(B, H, V, DEPTH, **inputs)
    res = bass_utils.run_bass_kernel_spmd(
        nc, in_maps, core_ids=list(range(N_CORES)), trace=trace)

    L = 1 << DEPTH
    Bl = B // N_CORES
    Vs = V // N_CORES
    # leaf column order per core: col = jj*Bl + e with jj = bitrev(true leaf)
    rev = np.array([int(format(t, f"0{DEPTH}b")[::-1], 2) for t in range(L)])
    # log-softmax denominator: sum the per-shard exp-sums across cores
    s_tot = np.zeros((B * L,), np.float64)
    for c in range(N_CORES):
        s = res.results[c]["s_out"]                  # [128, RT]
        s_tot += s.T.reshape(-1).astype(np.float64)  # row = rt*128 + p
    lse = np.log(s_tot).astype(np.float32)           # [B*L] in device row order
    lse = lse.reshape(N_CORES, L, Bl).transpose(0, 2, 1).reshape(B, L)[:, rev]
    full = np.empty((B, L, V), np.float32)
    for c in range(N_CORES):
        o = res.results[c]["out"]                    # [B*L, Vs]
        o = o.reshape(N_CORES, L, Bl, Vs)            # [src_core, jj, e, v]
        o = o.transpose(0, 2, 1, 3).reshape(B, L, Vs)
        full[:, :, c * Vs:(c + 1) * Vs] = o[:, rev, :] - lse[:, :, None]
    return full, res


def kernel(**inputs):
    enc = np.asarray(inputs["encoding"], np.float32)
    B, H = enc.shape
    V = np.asarray(inputs["W_out"]).shape[0]
    DEPTH = int(inputs["depth"])
    args = {k: np.asarray(v, np.float32) for k, v in inputs.items() if k != "depth"}
    full, _ = _run(B, H, V, DEPTH, args)
    return full



# revision 3
# speedup vs baseline: 1.2198x; 1.2198x over previous
"""DecoderTreeRNN Trainium2 kernel (8 NeuronCores, single SPMD launch).

Sharding: hybrid 2-way vocab x 4-way rows. Tree expansion is data-parallel
over batch (8 examples/core). Leaf exchange is a PAIR collective only:
cores {2p, 2p+1} swap their 256 leaf rows, then each projects the pair's
512 rows onto its 16000-column vocab half. The per-row exp-sums of the two
halves are added on the host, which folds -log(sum) into the unshard pass.

The pair exchange is a ReduceScatter(add) whose input is the core's own
leaves duplicated into both rank slots: every core receives mine+partner
in the SAME local buffer (no rank-dependent slot indexing, which SPMD code
could not express), and one vector subtract recovers the partner's leaves.
The collective runs entirely under the projection of the core's own rows,
which need no communication, so its ~35us fixed cost leaves the critical
path.

  - Tree: GRU states transposed [H, nodes] bf16; gh = WhhT.T @ hT with fp8
    weight tiles stationary on the PE. Biases fold in via free-dim-broadcast
    adds (VectorE); sigmoid/tanh on ScalarE; 4-wide m-tile slabs. Children
    concatenate [left | right]; bit-reversed leaf order undone on the host.
    The last level lands in an outer-pool tile that survives into the
    projection, so own leaf rows never round-trip through DRAM.
  - Projection: fp8 DoubleRow matmuls (K=256/tile), stationary leaf tiles,
    streaming vocab columns, one PSUM bank per 500-col chunk. Vocab bias
    (bf16, host-pre-broadcast) is added in the PSUM->SBUF copy; exp+row-sum
    is fused on ScalarE via accum_out. Logits stream out in bf16: the host
    computes log-softmax from the same quantized logits, so the systematic
    part of the quantization cancels. W half 1 (8.2MB) is resident from
    kernel start; half 2 loads into the SBUF the tree frees, its DMA parts
    interleaved between the own-row segments that cover it.
"""

import sys

for _p in ("/opt/trn_rl_repo",):
    if _p not in sys.path:
        sys.path.append(_p)

import numpy as np
import ml_dtypes

import concourse.bass as bass
from concourse import bacc, tile, mybir
from concourse import bass_utils
from contextlib import ExitStack

BF16 = mybir.dt.bfloat16
F32 = mybir.dt.float32
AF = mybir.ActivationFunctionType
ALU = mybir.AluOpType
BFNP = ml_dtypes.bfloat16
FP8 = mybir.dt.float8e4

N_CORES = 8
VP = 2                   # vocab-parallel ways
CW = 500                 # vocab chunk width (<=512 f32 psum bank)


def _build(B, H, V, DEPTH):
    KT = H // 128            # contraction tiles
    KT2 = KT // 2            # DoubleRow k-tiles (K=256 each)
    Bl = B // N_CORES        # examples per core
    L = 1 << DEPTH           # leaves per example
    NLOC = Bl * L            # local leaf count
    ROWS = 2 * NLOC          # rows projected per core (own + partner)
    RT = ROWS // 128         # row tiles
    RTH = RT // 2            # row tiles per half (own / partner)
    Vs = V // VP             # vocab shard
    NCH = Vs // CW           # chunks per shard
    NCG = NCH // 8           # chunk groups (8 psum banks each)
    Vh = Vs // 2             # W resident half (columns)
    SG = min(4, KT)          # m-tiles per gate slab
    NSL = KT // SG           # slabs per gate
    assert B % N_CORES == 0 and H % 128 == 0 and V % VP == 0
    assert Vs % (8 * CW) == 0 and ROWS % 128 == 0
    assert SG * 128 <= 512 and NCG % 2 == 0

    nc = bacc.Bacc("TRN2", target_bir_lowering=False, debug=False,
                   num_devices=N_CORES, dynamic_dma_scratch_size=2048)

    # ---------------- DRAM I/O ----------------
    encT = nc.dram_tensor("encT", [H, Bl], BF16, kind="ExternalInput")
    wt_d, wb_d, bih2_d = {}, {}, {}
    for s in "lr":
        wt_d[s] = nc.dram_tensor(f"wt_{s}", [H, 3 * H], FP8, kind="ExternalInput")
        wb_d[s] = nc.dram_tensor(f"wb_{s}", [128, 3 * KT], F32, kind="ExternalInput")
        bih2_d[s] = nc.dram_tensor(f"bih2_{s}", [128, KT], F32,
                                   kind="ExternalInput")
    wo1_d = nc.dram_tensor("woT1", [128, KT2, 2, Vh], FP8, kind="ExternalInput")
    wo2_d = nc.dram_tensor("woT2", [128, KT2, 2, Vh], FP8, kind="ExternalInput")
    bo_d = nc.dram_tensor("bo", [128, Vs], BF16, kind="ExternalInput")
    out_d = nc.dram_tensor("out", [ROWS, Vs], BF16, kind="ExternalOutput")
    s_out_d = nc.dram_tensor("s_out", [128, RT], F32, kind="ExternalOutput")

    # pair ReduceScatter operands (both Local: collectives cannot read
    # Shared, and <=4-core groups cannot write Shared)
    bounce2 = nc.dram_tensor("bounce2", [2 * H, NLOC], BF16, kind="Internal")
    rs_d = nc.dram_tensor("rs_sum", [H, NLOC], BF16, kind="Internal")
    rg = [[2 * p, 2 * p + 1] for p in range(N_CORES // 2)]

    with tile.TileContext(nc) as tc, ExitStack() as ctx:
        wproj = ctx.enter_context(tc.tile_pool(name="wproj", bufs=1))
        cpool = ctx.enter_context(tc.tile_pool(name="const", bufs=1))
        lvp = ctx.enter_context(tc.tile_pool(name="leaves", bufs=1))

        # resident projection weights half 1 + bias; issued after the tree
        # weights below so the tree isn't starved at start.
        wo1_sb = wproj.tile([128, KT2, 2, Vh], FP8, tag="wo1", name="wo1")
        bo_sb = cpool.tile([128, Vs], BF16, tag="bo")
        lv_own_b = lvp.tile([128, KT, NLOC], BF16, tag="lvb", name="lv_own_b")
        lv_own8 = lvp.tile([128, KT, NLOC], FP8, tag="lv8", name="lv_own8")
        lv_par8 = lvp.tile([128, KT, NLOC], FP8, tag="lvp8", name="lv_par8")

        # ---------------- tree expansion ----------------
        with nc.named_scope("tree"):
            with tc.tile_pool(name="wtree", bufs=1) as wtp, \
                 tc.tile_pool(name="state", bufs=2) as stp, \
                 tc.tile_pool(name="gates", bufs=2) as gp, \
                 tc.tile_pool(name="pstree", bufs=8, space="PSUM") as pst:
                # latency-critical small inputs on the ACT ring so they don't
                # queue behind the big weight loads (SP ring is FIFO)
                cur = stp.tile([128, KT, Bl], BF16, tag="st")
                nc.scalar.dma_start(cur[:], encT.ap().rearrange("(k p) b -> p k b", k=KT))
                wt_sb, wb_sb, bih2_sb = {}, {}, {}
                for s in "lr":
                    wb_sb[s] = wtp.tile([128, 3 * KT], F32, tag=f"wb{s}", name=f"wb_sb_{s}")
                    nc.scalar.dma_start(wb_sb[s][:], wb_d[s].ap())
                    bih2_sb[s] = wtp.tile([128, KT], F32, tag=f"bi{s}", name=f"bih2_sb_{s}")
                    nc.scalar.dma_start(bih2_sb[s][:], bih2_d[s].ap())
                # weight loads in consumption order: side l, side r, then the
                # projection weights behind them
                for s in "lr":
                    eng = nc.sync if s == "l" else nc.scalar
                    wt_sb[s] = []
                    for k in range(KT):
                        t = wtp.tile([128, 3 * H], FP8, tag=f"wt{s}{k}")
                        eng.dma_start(t[:], wt_d[s].ap()[128 * k:128 * (k + 1), :])
                        wt_sb[s].append(t)
                nc.sync.dma_start(wo1_sb[:], wo1_d.ap())
                nc.scalar.dma_start(bo_sb[:], bo_d.ap())

                n = Bl
                for lvl in range(DEPTH):
                    last = lvl == DEPTH - 1
                    nxt = lv_own_b if last else stp.tile(
                        [128, KT, 2 * n], BF16, tag="st", name=f"nxt{lvl}")
                    for si, s in enumerate("lr"):
                        for sl in range(NSL):
                            ko0 = sl * SG
                            ps = {}
                            for gi, mb in (("r", ko0), ("z", KT + ko0), ("g", 2 * KT + ko0)):
                                p = pst.tile([128, SG, n], F32, tag="ps")
                                for mj in range(SG):
                                    m = mb + mj
                                    for k in range(KT):
                                        nc.tensor.matmul(
                                            p[:, mj, :],
                                            wt_sb[s][k][:, 128 * m:128 * (m + 1)],
                                            cur[:, k, :n],
                                            start=(k == 0), stop=(k == KT - 1))
                                ps[gi] = p
                            # biases folded in via free-dim-broadcast adds (DVE)
                            def _bias(mb_):
                                return wb_sb[s][:, mb_:mb_ + SG].unsqueeze(2)\
                                    .broadcast_to((128, SG, n))
                            y_r = gp.tile([128, SG, n], F32, tag="yr")
                            nc.vector.tensor_tensor(y_r[:], ps["r"][:], _bias(ko0), op=ALU.add)
                            r_t = gp.tile([128, SG, n], F32, tag="r")
                            nc.scalar.activation(r_t[:], y_r[:], AF.Sigmoid)
                            y_z = gp.tile([128, SG, n], F32, tag="yz")
                            nc.vector.tensor_tensor(y_z[:], ps["z"][:], _bias(KT + ko0), op=ALU.add)
                            z_t = gp.tile([128, SG, n], F32, tag="z")
                            nc.scalar.activation(z_t[:], y_z[:], AF.Sigmoid)
                            y_g = gp.tile([128, SG, n], F32, tag="yg")
                            nc.vector.tensor_tensor(y_g[:], ps["g"][:], _bias(2 * KT + ko0), op=ALU.add)
                            t_t = gp.tile([128, SG, n], F32, tag="t")
                            nc.vector.tensor_tensor(t_t[:], y_g[:], r_t[:], op=ALU.mult)
                            nc.vector.tensor_tensor(
                                t_t[:], t_t[:],
                                bih2_sb[s][:, ko0:ko0 + SG].unsqueeze(2)
                                .broadcast_to((128, SG, n)), op=ALU.add)
                            n_t = gp.tile([128, SG, n], F32, tag="n")
                            nc.scalar.activation(n_t[:], t_t[:], AF.Tanh)
                            u_t = gp.tile([128, SG, n], F32, tag="u")
                            nc.vector.scalar_tensor_tensor(
                                u_t[:], n_t[:], -1.0, cur[:, ko0:ko0 + SG, :n],
                                op0=ALU.mult, op1=ALU.add)  # u = h - n
                            nc.vector.tensor_tensor(u_t[:], u_t[:], z_t[:], op=ALU.mult)
                            nc.vector.tensor_tensor(
                                nxt[:, ko0:ko0 + SG, si * n:si * n + n],
                                u_t[:], n_t[:], op=ALU.add)
                    cur = nxt
                    n *= 2

                # duplicate own leaves into both rank slots of the RS input;
                # fp8 cast of own leaves for the projection runs on ScalarE
                # (ACT) in parallel with the DMA writes
                for k in range(KT):
                    eng = nc.sync if k % 2 == 0 else nc.scalar
                    eng.dma_start(bounce2[128 * k:128 * (k + 1), :],
                                  lv_own_b[:, k, :])
                    eng2 = nc.scalar if k % 2 == 0 else nc.sync
                    eng2.dma_start(bounce2[H + 128 * k:H + 128 * (k + 1), :],
                                   lv_own_b[:, k, :])
                nc.scalar.activation(lv_own8[:], lv_own_b[:], AF.Copy)

        # ---------------- pair leaf exchange ----------------
        with nc.named_scope("rs_pair"):
            nc.gpsimd.collective_compute(
                "ReduceScatter", ALU.add, replica_groups=rg,
                ins=[bounce2.ap()], outs=[rs_d.ap()])

        # ---------------- projection + log-softmax ----------------
        with nc.named_scope("proj"):
            with tc.tile_pool(name="wproj2", bufs=1) as wp2, \
                 tc.tile_pool(name="rsp", bufs=1) as rsp, \
                 tc.tile_pool(name="lgt", bufs=6) as lgp, \
                 tc.tile_pool(name="scr", bufs=6) as scp, \
                 tc.tile_pool(name="stats", bufs=1) as sp2, \
                 tc.tile_pool(name="psproj", bufs=8, space="PSUM") as psp:
                # W half 2 into the SBUF the tree just released; DMA in 4
                # parts interleaved between own-row segments (below) so the
                # output DMAs aren't stuck behind one huge FIFO entry
                wo2_sb = wp2.tile([128, KT2, 2, Vh], FP8, tag="wo2", name="wo2")
                WP = 4
                wo2_parts = [(wo2_sb[:, :, :, i * (Vh // WP):(i + 1) * (Vh // WP)],
                              wo2_d.ap()[:, :, :, i * (Vh // WP):(i + 1) * (Vh // WP)])
                             for i in range(WP)]

                s_all = sp2.tile([128, RT], F32, tag="sall", name="s_all")
                sp_rt = [sp2.tile([128, NCH], F32, tag=f"sp{r}", name=f"sp{r}")
                         for r in range(RT)]

                def seg(rt, cgs, src):
                    for cg in cgs:
                        pps = [psp.tile([128, CW], F32, tag="pp",
                                        name=f"pp{rt}_{cg}_{i}")
                               for i in range(8)]
                        c0 = 128 * (rt % RTH)
                        for k2 in range(KT2):
                            lhsT = src[:, 2 * k2:2 * k2 + 2, c0:c0 + 128]
                            for i in range(8):
                                nch = 8 * cg + i
                                wsb = wo1_sb if nch < NCH // 2 else wo2_sb
                                woff = CW * nch - (0 if nch < NCH // 2 else Vh)
                                nc.tensor.matmul(
                                    pps[i][:], lhsT,
                                    wsb[:, k2, :, woff:woff + CW],
                                    perf_mode=mybir.MatmulPerfMode.DoubleRow,
                                    start=(k2 == 0), stop=(k2 == KT2 - 1))
                        for i in range(8):
                            nch = 8 * cg + i
                            lg = lgp.tile([128, CW], BF16, tag="lg",
                                          name=f"lg{rt}_{nch}")
                            nc.vector.tensor_tensor(
                                lg[:], pps[i][:],
                                bo_sb[:, CW * nch:CW * nch + CW], op=ALU.add)
                            ex = scp.tile([128, CW], BF16, tag="exp",
                                          name=f"ex{rt}_{nch}")
                            nc.scalar.activation(
                                ex[:], lg[:], AF.Exp,
                                accum_out=sp_rt[rt][:, nch:nch + 1])
                            eng = nc.sync if nch % 2 == 0 else nc.scalar
                            eng.dma_start(
                                out_d.ap()[128 * rt:128 * (rt + 1),
                                           CW * nch:CW * nch + CW], lg[:])

                half1 = list(range(NCG // 2))
                half2 = list(range(NCG // 2, NCG))
                # own rows on resident W1: covers the W2 DMA + collective
                seg(0, half1, lv_own8)
                nc.sync.dma_start(*wo2_parts[0])
                nc.sync.dma_start(*wo2_parts[1])
                seg(1, half1, lv_own8)
                nc.sync.dma_start(*wo2_parts[2])
                nc.sync.dma_start(*wo2_parts[3])
                seg(0, half2, lv_own8)
                seg(1, half2, lv_own8)

                # partner rows: mine+partner arrived via the pair RS; one
                # subtract recovers the partner's leaves (fp8 out)
                rs_sb = rsp.tile([128, KT, NLOC], BF16, tag="rs", name="rs_sb")
                for k in range(KT):
                    eng = nc.sync if k % 2 == 0 else nc.scalar
                    eng.dma_start(rs_sb[:, k, :],
                                  rs_d.ap()[128 * k:128 * (k + 1), :])
                nc.vector.tensor_tensor(lv_par8[:], rs_sb[:], lv_own_b[:],
                                        op=ALU.subtract)

                seg(2, half1, lv_par8)
                seg(3, half1, lv_par8)
                seg(2, half2, lv_par8)
                seg(3, half2, lv_par8)

                for r in range(RT):
                    nc.vector.reduce_sum(s_all[:, r:r + 1], sp_rt[r][:],
                                         axis=mybir.AxisListType.X)
                nc.scalar.dma_start(s_out_d.ap()[:, :], s_all[:])

    nc.compile()
    return nc


_CACHE = {}


def _get(B, H, V, DEPTH):
    key = (B, H, V, DEPTH)
    if key not in _CACHE:
        _CACHE[key] = _build(B, H, V, DEPTH)
    return _CACHE[key]


def _pack_inputs(B, H, V, DEPTH, encoding, Whh_l, bih_l, bhh_l, Whh_r, bih_r,
                 bhh_r, W_out, b_out):
    """Host-side shard + transpose + cast. Returns in_maps for the 8 cores."""
    KT = H // 128
    KT2 = KT // 2
    Bl = B // N_CORES
    Vs = V // VP
    Vh = Vs // 2

    woT = np.ascontiguousarray(W_out.T).astype(np.float32)    # [H, V]
    encT = np.ascontiguousarray(encoding.T).astype(BFNP)      # [H, B]

    shared = {}
    for s, Whh, bih, bhh in (("l", Whh_l, bih_l, bhh_l), ("r", Whh_r, bih_r, bhh_r)):
        shared[f"wt_{s}"] = np.ascontiguousarray(Whh.T).astype(
            mybir.dt.np(FP8))  # [H, 3H] fp8: weight-load bound, not precision bound
        # bias row folded into the matmul: sigmoid gates get bih+bhh,
        # candidate gate gets bhh only (bih_n is added after the r-multiply)
        wb = np.concatenate([(bih + bhh)[:2 * H], bhh[2 * H:]])
        shared[f"wb_{s}"] = np.ascontiguousarray(
            wb.reshape(3 * KT, 128).T.astype(np.float32))
        shared[f"bih2_{s}"] = np.ascontiguousarray(
            bih[2 * H:].reshape(KT, 128).T.astype(np.float32))  # [128, KT]

    def pack_w(wcols):  # [H, Vh] -> [128, KT2, 2, Vh] fp8
        w = wcols.reshape(KT2, 2, 128, wcols.shape[1])
        return np.ascontiguousarray(w.transpose(2, 0, 1, 3)).astype(
            mybir.dt.np(FP8))

    in_maps = []
    for c in range(N_CORES):
        q = c % 2
        m = dict(shared)
        m["encT"] = np.ascontiguousarray(encT[:, c * Bl:(c + 1) * Bl])
        half = woT[:, q * Vs:(q + 1) * Vs]
        m["woT1"] = pack_w(half[:, :Vh])
        m["woT2"] = pack_w(half[:, Vh:])
        m["bo"] = np.ascontiguousarray(np.broadcast_to(
            b_out[q * Vs:(q + 1) * Vs].astype(BFNP), (128, Vs)))
        in_maps.append(m)
    return in_maps


def _run(B, H, V, DEPTH, inputs, trace=False, nc=None):
    if nc is None:
        nc = _get(B, H, V, DEPTH)
    in_maps = _pack_inputs(B, H, V, DEPTH, **inputs)
    res = bass_utils.run_bass_kernel_spmd(
        nc, in_maps, core_ids=list(range(N_CORES)), trace=trace)

    L = 1 << DEPTH
    Bl = B // N_CORES
    Vs = V // VP
    # leaf column order per core half: col = jj*Bl + e, jj = bitrev(leaf)
    rev = np.array([int(format(t, f"0{DEPTH}b")[::-1], 2) for t in range(L)])
    # device rows on core c: half 0 = own examples (core c), half 1 =
    # partner (core c^1); within a half: row = jj*Bl + e
    s_tot = np.zeros((B, L), np.float64)
    os = []
    for c in range(N_CORES):
        s = res.results[c]["s_out"]                  # [128, RT]
        s = s.T.reshape(2, L, Bl)                    # [half, jj, e]
        o = res.results[c]["out"].astype(np.float32) # [512, Vs] bf16
        o = o.reshape(2, L, Bl, Vs)
        os.append(o)
        for h in range(2):
            ex = c if h == 0 else c ^ 1
            # global b = ex*Bl + e ; true leaf l has jj = rev[l]
            s_tot[ex * Bl:(ex + 1) * Bl, :] += \
                s[h][rev, :].T.astype(np.float64)
    lse = np.log(s_tot).astype(np.float32)           # [B, L]
    full = np.empty((B, L, V), np.float32)
    for c in range(N_CORES):
        q = c % 2
        o = os[c]
        for h in range(2):
            ex = c if h == 0 else c ^ 1
            full[ex * Bl:(ex + 1) * Bl, :, q * Vs:(q + 1) * Vs] = \
                o[h][rev, :, :].transpose(1, 0, 2)
    full -= lse[:, :, None]
    return full, res


def kernel(**inputs):
    enc = np.asarray(inputs["encoding"], np.float32)
    B, H = enc.shape
    V = np.asarray(inputs["W_out"]).shape[0]
    DEPTH = int(inputs["depth"])
    args = {k: np.asarray(v, np.float32) for k, v in inputs.items() if k != "depth"}
    full, _ = _run(B, H, V, DEPTH, args)
    return full


# revision 7
# speedup vs baseline: 1.2962x; 1.0627x over previous
"""DecoderTreeRNN Trainium2 kernel (8 NeuronCores, single SPMD launch).

Sharding: hybrid 2-way vocab x 4-way rows. Tree expansion is data-parallel
over batch (8 examples/core). Leaf exchange is a PAIR collective only:
cores {2p, 2p+1} swap their 256 leaf rows, then each projects the pair's
512 rows onto its 16000-column vocab half. The per-row exp-sums of the two
halves are added on the host, which folds -log(sum) into the unshard pass.

The pair exchange is a ReduceScatter(add) whose input is the core's own
leaves duplicated into both rank slots: every core receives mine+partner
in the SAME local buffer (no rank-dependent slot indexing, which SPMD code
could not express), and one vector subtract recovers the partner's leaves.
The collective runs entirely under the projection of the core's own rows,
which need no communication, so its ~35us fixed cost leaves the critical
path.

  - Tree: GRU states transposed [H, nodes] bf16; gh = WhhT.T @ hT with fp8
    weight tiles stationary on the PE. Biases fold in via free-dim-broadcast
    adds (VectorE); sigmoid/tanh on ScalarE; 4-wide m-tile slabs. Children
    concatenate [left | right]; bit-reversed leaf order undone on the host.
    The last level lands in an outer-pool tile that survives into the
    projection, so own leaf rows never round-trip through DRAM.
  - Projection: fp8 DoubleRow matmuls (K=256/tile), stationary leaf tiles,
    streaming vocab columns, one PSUM bank per 500-col chunk. Vocab bias
    (bf16, host-pre-broadcast) is added in the PSUM->SBUF copy; exp+row-sum
    is fused on ScalarE via accum_out. Logits stream out in bf16: the host
    computes log-softmax from the same quantized logits, so the systematic
    part of the quantization cancels. W half 1 (8.2MB) is resident from
    kernel start; half 2 loads into the SBUF the tree frees, its DMA parts
    interleaved between the own-row segments that cover it.
"""

import sys

for _p in ("/opt/trn_rl_repo",):
    if _p not in sys.path:
        sys.path.append(_p)

import numpy as np
import ml_dtypes

import concourse.bass as bass
from concourse import bacc, tile, mybir
from concourse import bass_utils
from contextlib import ExitStack

BF16 = mybir.dt.bfloat16
F32 = mybir.dt.float32
AF = mybir.ActivationFunctionType
ALU = mybir.AluOpType
BFNP = ml_dtypes.bfloat16
FP8 = mybir.dt.float8e4

N_CORES = 8
VP = 2                   # vocab-parallel ways
CW = 500                 # vocab chunk width (<=512 f32 psum bank)


def _build(B, H, V, DEPTH):
    KT = H // 128            # contraction tiles
    KT2 = KT // 2            # DoubleRow k-tiles (K=256 each)
    Bl = B // N_CORES        # examples per core
    L = 1 << DEPTH           # leaves per example
    NLOC = Bl * L            # local leaf count
    ROWS = 2 * NLOC          # rows projected per core (own + partner)
    RT = ROWS // 128         # row tiles
    RTH = RT // 2            # row tiles per half (own / partner)
    Vs = V // VP             # vocab shard
    NCH = Vs // CW           # chunks per shard
    NCG = NCH // 8           # chunk groups (8 psum banks each)
    Vh = Vs // 2             # W resident half (columns)
    SG = min(4, KT)          # m-tiles per gate slab
    NSL = KT // SG           # slabs per gate
    assert B % N_CORES == 0 and H % 128 == 0 and V % VP == 0
    assert Vs % (8 * CW) == 0 and ROWS % 128 == 0
    assert SG * 128 <= 512 and NCG % 2 == 0

    nc = bacc.Bacc("TRN2", target_bir_lowering=False, debug=False,
                   num_devices=N_CORES, dynamic_dma_scratch_size=2048)

    # ---------------- DRAM I/O ----------------
    encT = nc.dram_tensor("encT", [H, Bl], BF16, kind="ExternalInput")
    wt_d, wb_d, bih2_d = {}, {}, {}
    for s in "lr":
        wt_d[s] = nc.dram_tensor(f"wt_{s}", [H, 3 * H], FP8, kind="ExternalInput")
        wb_d[s] = nc.dram_tensor(f"wb_{s}", [128, 3 * KT], F32, kind="ExternalInput")
        bih2_d[s] = nc.dram_tensor(f"bih2_{s}", [128, KT], F32,
                                   kind="ExternalInput")
    wo1_d = nc.dram_tensor("woT1", [128, KT2, 2, Vh], FP8, kind="ExternalInput")
    wo2_d = nc.dram_tensor("woT2", [128, KT2, 2, Vh], FP8, kind="ExternalInput")
    bo_d = nc.dram_tensor("bo", [128, Vs], BF16, kind="ExternalInput")
    out_d = nc.dram_tensor("out", [ROWS, Vs], BF16, kind="ExternalOutput")
    s_out_d = nc.dram_tensor("s_out", [128, RT], F32, kind="ExternalOutput")

    # pair ReduceScatter operands (both Local: collectives cannot read
    # Shared, and <=4-core groups cannot write Shared)
    bounce2 = nc.dram_tensor("bounce2", [2 * H, NLOC], BF16, kind="Internal")
    rs_d = nc.dram_tensor("rs_sum", [H, NLOC], BF16, kind="Internal")
    rg = [[2 * p, 2 * p + 1] for p in range(N_CORES // 2)]

    with tile.TileContext(nc) as tc, ExitStack() as ctx:
        wproj = ctx.enter_context(tc.tile_pool(name="wproj", bufs=1))
        cpool = ctx.enter_context(tc.tile_pool(name="const", bufs=1))
        lvp = ctx.enter_context(tc.tile_pool(name="leaves", bufs=1))

        # resident projection weights half 1 + bias; issued after the tree
        # weights below so the tree isn't starved at start.
        wo1_sb = wproj.tile([128, KT2, 2, Vh], FP8, tag="wo1", name="wo1")
        bo_sb = cpool.tile([128, Vs], BF16, tag="bo")
        lv_own_b = lvp.tile([128, KT, NLOC], BF16, tag="lvb", name="lv_own_b")
        lv_own8 = lvp.tile([128, KT, NLOC], FP8, tag="lv8", name="lv_own8")
        lv_par8 = lvp.tile([128, KT, NLOC], FP8, tag="lvp8", name="lv_par8")

        # ---------------- tree expansion ----------------
        with nc.named_scope("tree"):
            with tc.tile_pool(name="wtree", bufs=1) as wtp, \
                 tc.tile_pool(name="state", bufs=2) as stp, \
                 tc.tile_pool(name="gates", bufs=2) as gp, \
                 tc.tile_pool(name="pstree", bufs=8, space="PSUM") as pst:
                # latency-critical small inputs on the ACT ring so they don't
                # queue behind the big weight loads (SP ring is FIFO)
                cur = stp.tile([128, KT, Bl], BF16, tag="st")
                nc.scalar.dma_start(cur[:], encT.ap().rearrange("(k p) b -> p k b", k=KT))
                wt_sb, wb_sb, bih2_sb = {}, {}, {}
                for s in "lr":
                    wb_sb[s] = wtp.tile([128, 3 * KT], F32, tag=f"wb{s}", name=f"wb_sb_{s}")
                    nc.scalar.dma_start(wb_sb[s][:], wb_d[s].ap())
                    bih2_sb[s] = wtp.tile([128, KT], F32, tag=f"bi{s}", name=f"bih2_sb_{s}")
                    nc.scalar.dma_start(bih2_sb[s][:], bih2_d[s].ap())
                # weight loads in consumption order: side l, side r, then the
                # projection weights behind them
                for s in "lr":
                    eng = nc.sync if s == "l" else nc.scalar
                    wt_sb[s] = []
                    for k in range(KT):
                        t = wtp.tile([128, 3 * H], FP8, tag=f"wt{s}{k}")
                        eng.dma_start(t[:], wt_d[s].ap()[128 * k:128 * (k + 1), :])
                        wt_sb[s].append(t)
                nc.sync.dma_start(wo1_sb[:], wo1_d.ap())
                nc.scalar.dma_start(bo_sb[:], bo_d.ap())

                n = Bl
                for lvl in range(DEPTH):
                    last = lvl == DEPTH - 1
                    nxt = lv_own_b if last else stp.tile(
                        [128, KT, 2 * n], BF16, tag="st", name=f"nxt{lvl}")
                    for si, s in enumerate("lr"):
                        for sl in range(NSL):
                            ko0 = sl * SG
                            ps = {}
                            for gi, mb in (("r", ko0), ("z", KT + ko0), ("g", 2 * KT + ko0)):
                                p = pst.tile([128, SG, n], F32, tag="ps")
                                for mj in range(SG):
                                    m = mb + mj
                                    for k in range(KT):
                                        nc.tensor.matmul(
                                            p[:, mj, :],
                                            wt_sb[s][k][:, 128 * m:128 * (m + 1)],
                                            cur[:, k, :n],
                                            start=(k == 0), stop=(k == KT - 1))
                                ps[gi] = p
                            # biases folded in via free-dim-broadcast adds (DVE)
                            def _bias(mb_):
                                return wb_sb[s][:, mb_:mb_ + SG].unsqueeze(2)\
                                    .broadcast_to((128, SG, n))
                            y_r = gp.tile([128, SG, n], F32, tag="yr")
                            nc.vector.tensor_tensor(y_r[:], ps["r"][:], _bias(ko0), op=ALU.add)
                            r_t = gp.tile([128, SG, n], F32, tag="r")
                            nc.scalar.activation(r_t[:], y_r[:], AF.Sigmoid)
                            y_z = gp.tile([128, SG, n], F32, tag="yz")
                            nc.vector.tensor_tensor(y_z[:], ps["z"][:], _bias(KT + ko0), op=ALU.add)
                            z_t = gp.tile([128, SG, n], F32, tag="z")
                            nc.scalar.activation(z_t[:], y_z[:], AF.Sigmoid)
                            y_g = gp.tile([128, SG, n], F32, tag="yg")
                            nc.vector.tensor_tensor(y_g[:], ps["g"][:], _bias(2 * KT + ko0), op=ALU.add)
                            t_t = gp.tile([128, SG, n], F32, tag="t")
                            nc.vector.tensor_tensor(t_t[:], y_g[:], r_t[:], op=ALU.mult)
                            nc.vector.tensor_tensor(
                                t_t[:], t_t[:],
                                bih2_sb[s][:, ko0:ko0 + SG].unsqueeze(2)
                                .broadcast_to((128, SG, n)), op=ALU.add)
                            n_t = gp.tile([128, SG, n], F32, tag="n")
                            nc.scalar.activation(n_t[:], t_t[:], AF.Tanh)
                            u_t = gp.tile([128, SG, n], F32, tag="u")
                            nc.vector.scalar_tensor_tensor(
                                u_t[:], n_t[:], -1.0, cur[:, ko0:ko0 + SG, :n],
                                op0=ALU.mult, op1=ALU.add)  # u = h - n
                            nc.vector.tensor_tensor(u_t[:], u_t[:], z_t[:], op=ALU.mult)
                            nc.vector.tensor_tensor(
                                nxt[:, ko0:ko0 + SG, si * n:si * n + n],
                                u_t[:], n_t[:], op=ALU.add)
                    cur = nxt
                    n *= 2

                # fp8 cast first: it gates the projection start (ScalarE);
                # the bounce writes only gate the collective, which has the
                # whole own-rows phase of slack
                nc.scalar.activation(lv_own8[:], lv_own_b[:], AF.Copy)
                # duplicate own leaves into both rank slots of the RS input
                for k in range(KT):
                    eng = nc.sync if k % 2 == 0 else nc.scalar
                    eng.dma_start(bounce2[128 * k:128 * (k + 1), :],
                                  lv_own_b[:, k, :])
                    eng2 = nc.scalar if k % 2 == 0 else nc.sync
                    eng2.dma_start(bounce2[H + 128 * k:H + 128 * (k + 1), :],
                                   lv_own_b[:, k, :])

        # ---------------- pair leaf exchange ----------------
        with nc.named_scope("rs_pair"):
            nc.gpsimd.collective_compute(
                "ReduceScatter", ALU.add, replica_groups=rg,
                ins=[bounce2.ap()], outs=[rs_d.ap()])

        # ---------------- projection + log-softmax ----------------
        with nc.named_scope("proj"):
            with tc.tile_pool(name="wproj2", bufs=1) as wp2, \
                 tc.tile_pool(name="rsp", bufs=1) as rsp, \
                 tc.tile_pool(name="lgt", bufs=6) as lgp, \
                 tc.tile_pool(name="scr", bufs=6) as scp, \
                 tc.tile_pool(name="stats", bufs=1) as sp2, \
                 tc.tile_pool(name="psproj", bufs=8, space="PSUM") as psp:
                # W half 2 into the SBUF the tree just released; DMA in 4
                # parts interleaved between own-row segments (below) so the
                # output DMAs aren't stuck behind one huge FIFO entry
                wo2_sb = wp2.tile([128, KT2, 2, Vh], FP8, tag="wo2", name="wo2")
                WP = 4
                wo2_parts = [(wo2_sb[:, :, :, i * (Vh // WP):(i + 1) * (Vh // WP)],
                              wo2_d.ap()[:, :, :, i * (Vh // WP):(i + 1) * (Vh // WP)])
                             for i in range(WP)]

                s_all = sp2.tile([128, RT], F32, tag="sall", name="s_all")
                sp_rt = [sp2.tile([128, NCH], F32, tag=f"sp{r}", name=f"sp{r}")
                         for r in range(RT)]

                def seg(rt, cgs, src):
                    for cg in cgs:
                        pps = [psp.tile([128, CW], F32, tag="pp",
                                        name=f"pp{rt}_{cg}_{i}")
                               for i in range(8)]
                        c0 = 128 * (rt % RTH)
                        for k2 in range(KT2):
                            lhsT = src[:, 2 * k2:2 * k2 + 2, c0:c0 + 128]
                            for i in range(8):
                                nch = 8 * cg + i
                                wsb = wo1_sb if nch < NCH // 2 else wo2_sb
                                woff = CW * nch - (0 if nch < NCH // 2 else Vh)
                                nc.tensor.matmul(
                                    pps[i][:], lhsT,
                                    wsb[:, k2, :, woff:woff + CW],
                                    perf_mode=mybir.MatmulPerfMode.DoubleRow,
                                    start=(k2 == 0), stop=(k2 == KT2 - 1))
                        for i in range(8):
                            nch = 8 * cg + i
                            lg = lgp.tile([128, CW], BF16, tag="lg",
                                          name=f"lg{rt}_{nch}")
                            nc.vector.tensor_tensor(
                                lg[:], pps[i][:],
                                bo_sb[:, CW * nch:CW * nch + CW], op=ALU.add)
                            ex = scp.tile([128, CW], BF16, tag="exp",
                                          name=f"ex{rt}_{nch}")
                            nc.scalar.activation(
                                ex[:], lg[:], AF.Exp,
                                accum_out=sp_rt[rt][:, nch:nch + 1])
                            # outputs all on the SP ring: ScalarE is the
                            # projection's critical engine (exp+accum), so
                            # don't spend its issue slots on bulk DMA
                            nc.sync.dma_start(
                                out_d.ap()[128 * rt:128 * (rt + 1),
                                           CW * nch:CW * nch + CW], lg[:])

                # W2 parts on the ACT ring right away: both rings idle at
                # tree end, and the own-row segments cover the transfer
                for i in range(WP):
                    nc.scalar.dma_start(*wo2_parts[i])

                half1 = list(range(NCG // 2))
                half2 = list(range(NCG // 2, NCG))
                # own rows on resident W1: covers the W2 DMA + collective
                seg(0, half1, lv_own8)
                seg(1, half1, lv_own8)

                # partner rows: mine+partner arrived via the pair RS; one
                # subtract recovers the partner's leaves (fp8 out). The
                # whole chain (read-back + subtract) runs on gpsimd, which
                # just finished the collective and is otherwise idle, so no
                # critical engine ever blocks on the RS semaphore.
                rs_sb = rsp.tile([128, KT, NLOC], BF16, tag="rs", name="rs_sb")
                nc.gpsimd.dma_start(
                    rs_sb[:], rs_d.ap().rearrange("(k p) j -> p k j", k=KT))
                nc.gpsimd.tensor_tensor(lv_par8[:], rs_sb[:], lv_own_b[:],
                                        op=ALU.subtract)

                seg(0, half2, lv_own8)
                seg(1, half2, lv_own8)
                seg(2, half1, lv_par8)
                seg(3, half1, lv_par8)
                seg(2, half2, lv_par8)
                seg(3, half2, lv_par8)

                for r in range(RT):
                    nc.vector.reduce_sum(s_all[:, r:r + 1], sp_rt[r][:],
                                         axis=mybir.AxisListType.X)
                nc.scalar.dma_start(s_out_d.ap()[:, :], s_all[:])

    nc.compile()
    return nc


_CACHE = {}


def _get(B, H, V, DEPTH):
    key = (B, H, V, DEPTH)
    if key not in _CACHE:
        _CACHE[key] = _build(B, H, V, DEPTH)
    return _CACHE[key]


def _pack_inputs(B, H, V, DEPTH, encoding, Whh_l, bih_l, bhh_l, Whh_r, bih_r,
                 bhh_r, W_out, b_out):
    """Host-side shard + transpose + cast. Returns in_maps for the 8 cores."""
    KT = H // 128
    KT2 = KT // 2
    Bl = B // N_CORES
    Vs = V // VP
    Vh = Vs // 2

    woT = np.ascontiguousarray(W_out.T).astype(np.float32)    # [H, V]
    encT = np.ascontiguousarray(encoding.T).astype(BFNP)      # [H, B]

    shared = {}
    for s, Whh, bih, bhh in (("l", Whh_l, bih_l, bhh_l), ("r", Whh_r, bih_r, bhh_r)):
        shared[f"wt_{s}"] = np.ascontiguousarray(Whh.T).astype(
            mybir.dt.np(FP8))  # [H, 3H] fp8: weight-load bound, not precision bound
        # bias row folded into the matmul: sigmoid gates get bih+bhh,
        # candidate gate gets bhh only (bih_n is added after the r-multiply)
        wb = np.concatenate([(bih + bhh)[:2 * H], bhh[2 * H:]])
        shared[f"wb_{s}"] = np.ascontiguousarray(
            wb.reshape(3 * KT, 128).T.astype(np.float32))
        shared[f"bih2_{s}"] = np.ascontiguousarray(
            bih[2 * H:].reshape(KT, 128).T.astype(np.float32))  # [128, KT]

    def pack_w(wcols):  # [H, Vh] -> [128, KT2, 2, Vh] fp8
        w = wcols.reshape(KT2, 2, 128, wcols.shape[1])
        return np.ascontiguousarray(w.transpose(2, 0, 1, 3)).astype(
            mybir.dt.np(FP8))

    in_maps = []
    for c in range(N_CORES):
        q = c % 2
        m = dict(shared)
        m["encT"] = np.ascontiguousarray(encT[:, c * Bl:(c + 1) * Bl])
        half = woT[:, q * Vs:(q + 1) * Vs]
        m["woT1"] = pack_w(half[:, :Vh])
        m["woT2"] = pack_w(half[:, Vh:])
        m["bo"] = np.ascontiguousarray(np.broadcast_to(
            b_out[q * Vs:(q + 1) * Vs].astype(BFNP), (128, Vs)))
        in_maps.append(m)
    return in_maps


def _run(B, H, V, DEPTH, inputs, trace=False, nc=None):
    if nc is None:
        nc = _get(B, H, V, DEPTH)
    in_maps = _pack_inputs(B, H, V, DEPTH, **inputs)
    res = bass_utils.run_bass_kernel_spmd(
        nc, in_maps, core_ids=list(range(N_CORES)), trace=trace)

    L = 1 << DEPTH
    Bl = B // N_CORES
    Vs = V // VP
    # leaf column order per core half: col = jj*Bl + e, jj = bitrev(leaf)
    rev = np.array([int(format(t, f"0{DEPTH}b")[::-1], 2) for t in range(L)])
    # device rows on core c: half 0 = own examples (core c), half 1 =
    # partner (core c^1); within a half: row = jj*Bl + e
    s_tot = np.zeros((B, L), np.float64)
    os = []
    for c in range(N_CORES):
        s = res.results[c]["s_out"]                  # [128, RT]
        s = s.T.reshape(2, L, Bl)                    # [half, jj, e]
        o = res.results[c]["out"].astype(np.float32) # [512, Vs] bf16
        o = o.reshape(2, L, Bl, Vs)
        os.append(o)
        for h in range(2):
            ex = c if h == 0 else c ^ 1
            # global b = ex*Bl + e ; true leaf l has jj = rev[l]
            s_tot[ex * Bl:(ex + 1) * Bl, :] += \
                s[h][rev, :].T.astype(np.float64)
    lse = np.log(s_tot).astype(np.float32)           # [B, L]
    full = np.empty((B, L, V), np.float32)
    for c in range(N_CORES):
        q = c % 2
        o = os[c]
        for h in range(2):
            ex = c if h == 0 else c ^ 1
            full[ex * Bl:(ex + 1) * Bl, :, q * Vs:(q + 1) * Vs] = \
                o[h][rev, :, :].transpose(1, 0, 2)
    full -= lse[:, :, None]
    return full, res


def kernel(**inputs):
    enc = np.asarray(inputs["encoding"], np.float32)
    B, H = enc.shape
    V = np.asarray(inputs["W_out"]).shape[0]
    DEPTH = int(inputs["depth"])
    args = {k: np.asarray(v, np.float32) for k, v in inputs.items() if k != "depth"}
    full, _ = _run(B, H, V, DEPTH, args)
    return full
